# revision 4
# baseline (speedup 1.0000x reference)
"""Trainium2 Bass kernel for nn_MultiHeadAttention (B=2, S=2048, D=768, H=12).

Sharding: (batch, head-group) across 8 cores — core c handles batch c//4 and
heads 3*(c%4) .. 3*(c%4)+2 (Megatron-style column/row-parallel projections).
Returns (out [2,2048,768], attn [2,12,2048,2048]) like the reference.
"""

import math
import numpy as np

import concourse.bass as bass
import concourse.tile as tile
from concourse import bacc, mybir
from concourse.bass_utils import run_bass_kernel_spmd

B, S, D, H = 2, 2048, 768, 12
DEPTH = D // H          # 64
N_CORES = 8
HPC = H // 4            # 3 heads per core
DL = HPC * DEPTH        # 192 local head dims per core
QB = 1024               # q-block width for the transposed pass
NQB = S // QB           # 2
NKC = S // 128          # 16 k-chunks
NST = S // 128          # 16 s-tiles
SCALE = 1.0 / math.sqrt(DEPTH)  # 0.125

dt = mybir.dt
AF = mybir.ActivationFunctionType

_PROGRAM_CACHE = {}


def build_program(reps: int = 1):
    """Build the SPMD Bacc program (same program on all 8 cores)."""
    nc = bacc.Bacc("TRN2", target_bir_lowering=False, debug=False,
                   num_devices=N_CORES)

    f32, f32r = dt.float32, dt.float32r

    # ---- external I/O (per-core shapes) ----
    xqT = nc.dram_tensor("xqT", [D, S], f32, kind="ExternalInput").ap()
    xkT = nc.dram_tensor("xkT", [D, S], f32, kind="ExternalInput").ap()
    xvT = nc.dram_tensor("xvT", [D, S], f32, kind="ExternalInput").ap()
    wqT = nc.dram_tensor("wqT", [D, DL], f32, kind="ExternalInput").ap()
    wkT = nc.dram_tensor("wkT", [D, DL], f32, kind="ExternalInput").ap()
    wvT = nc.dram_tensor("wvT", [D, DL], f32, kind="ExternalInput").ap()
    bq = nc.dram_tensor("bq", [DL], f32, kind="ExternalInput").ap()
    bk = nc.dram_tensor("bk", [DL], f32, kind="ExternalInput").ap()
    bv = nc.dram_tensor("bv", [DL], f32, kind="ExternalInput").ap()
    woT = nc.dram_tensor("woT", [DL, D], f32, kind="ExternalInput").ap()
    wob = nc.dram_tensor("wob", [D], f32, kind="ExternalInput").ap()

    attn_out = nc.dram_tensor("attn_out", [HPC, S, S], f32,
                              kind="ExternalOutput").ap()
    out_slice = nc.dram_tensor("out_slice", [S // 4, D], f32,
                               kind="ExternalOutput").ap()

    with tile.TileContext(nc) as tc:
        _trace_body(nc, tc, xqT, xkT, xvT, wqT, wkT, wvT, bq, bk, bv,
                    woT, wob, attn_out, out_slice, reps)

    nc.compile()
    return nc


def _trace_body(nc, tc, xqT, xkT, xvT, wqT, wkT, wvT, bq, bk, bv,
                woT, wob, attn_out, out_slice, reps):
    f32, f32r = dt.float32, dt.float32r

    from contextlib import ExitStack
    es = ExitStack()
    with es:
        # ---------- persistent pools ----------
        const_pool = es.enter_context(tc.tile_pool(name="const", bufs=1))
        wpool = es.enter_context(tc.tile_pool(name="weights", bufs=1))
        qk_pool = es.enter_context(tc.tile_pool(name="qkT", bufs=1))
        v_pool = es.enter_context(tc.tile_pool(name="vaug", bufs=1))
        dram = es.enter_context(tc.tile_pool(name="dram", bufs=1, space="DRAM"))

        # ---------- constants ----------
        ones_col = const_pool.tile([128, 1], f32)       # per-partition 1.0
        nc.gpsimd.memset(ones_col[:], 1.0)
        ones_row32 = const_pool.tile([1, 128], f32)
        nc.gpsimd.memset(ones_row32[:], 1.0)
        ones_row = const_pool.tile([1, 128], f32r)
        nc.vector.tensor_copy(ones_row[:], ones_row32[:])
        ident = const_pool.tile([1, 1], f32)            # identity for 1xN transpose
        nc.gpsimd.memset(ident[:], 1.0)

        # ---------- load + round weights ----------
        # wqT/wkT/wvT [768, 192] -> 6 sbuf tiles [128, 192] each, fp32r
        w_r = {}
        wo_r = []
        with tc.tile_pool(name="wstage", bufs=2) as wstage:
            for name, wt in (("q", wqT), ("k", wkT), ("v", wvT)):
                wt_t = wt.rearrange("(j p) d -> j p d", p=128)
                big = wpool.tile([128, 6 * DL], f32r, tag=f"w{name}")
                for j in range(6):
                    stg = wstage.tile([128, DL], f32, tag="wstg")
                    nc.sync.dma_start(stg[:], wt_t[j])
                    nc.vector.tensor_copy(big[:, j * DL:(j + 1) * DL], stg[:])
                w_r[name] = big
            # woT [192, 768] -> 3 tiles [64, 768] fp32r
            woT_t = woT.rearrange("(h p) d -> h p d", p=DEPTH)
            for h in range(HPC):
                stg = wstage.tile([64, D], f32, tag="wostg")
                nc.sync.dma_start(stg[:], woT_t[h])
                t = wpool.tile([64, D], f32r, tag=f"wo{h}")
                nc.vector.tensor_copy(t[:], stg[:])
                wo_r.append(t)
        # biases bq/bk -> [64, 3] (column h = head h bias)
        bq_sb = const_pool.tile([64, HPC], f32)
        bk_sb = const_pool.tile([64, HPC], f32)
        nc.sync.dma_start(bq_sb[:], bq.rearrange("(h d) -> d h", d=DEPTH))
        nc.sync.dma_start(bk_sb[:], bk.rearrange("(h d) -> d h", d=DEPTH))
        # bv -> broadcast [128, 192];  wob -> broadcast [128, 768]
        brow = const_pool.tile([1, DL], f32)
        nc.sync.dma_start(brow[:], bv[None, :])
        brow_r = const_pool.tile([1, DL], f32r)
        nc.vector.tensor_copy(brow_r[:], brow[:])
        wrow = const_pool.tile([1, D], f32)
        nc.sync.dma_start(wrow[:], wob[None, :])
        wrow_r = const_pool.tile([1, D], f32r)
        nc.vector.tensor_copy(wrow_r[:], wrow[:])

        bv_bc = const_pool.tile([128, DL], f32)
        wob_bc = const_pool.tile([128, D], f32)
        with tc.tile_pool(name="ps_setup", bufs=2, space="PSUM") as ps_setup:
            p = ps_setup.tile([128, DL], f32, tag="bc")
            nc.tensor.matmul(p[:], ones_row[:], brow_r[:], start=True, stop=True)
            nc.vector.tensor_copy(bv_bc[:], p[:])
            p2 = ps_setup.tile([128, D], f32, tag="bc2")
            nc.tensor.matmul(p2[:, 0:512], ones_row[:], wrow_r[:, 0:512],
                             start=True, stop=True)
            nc.tensor.matmul(p2[:, 512:768], ones_row[:], wrow_r[:, 512:768],
                             start=True, stop=True)
            nc.vector.tensor_copy(wob_bc[:], p2[:])

        # ---------- persistent data tiles ----------
        # QhT / KhT: [64, 2048] fp32r per head
        qT = [qk_pool.tile([64, S], f32r, tag=f"qT{h}", name=f"qT{h}") for h in range(HPC)]
        kT = [qk_pool.tile([64, S], f32r, tag=f"kT{h}", name=f"kT{h}") for h in range(HPC)]
        # V augmented: per head [128, 16*65] (k-chunk kc at cols kc*65..+65)
        vaug = [v_pool.tile([128, NKC * (DEPTH + 1)], f32r, tag=f"va{h}", name=f"va{h}")
                for h in range(HPC)]
        # DRAM scratch for the output projection + reduce-scatter
        partial = dram.tile([S, D], f32)
        rs_out = dram.tile([S // 4, D], f32)

        def body(skip_rs=False):
            ctx_stack = ExitStack()
            # ================= Phase A: projections =================
            with tc.tile_pool(name="xstage", bufs=2) as xstage, \
                 tc.tile_pool(name="xr", bufs=1) as xr, \
                 tc.tile_pool(name="ps_a", bufs=3, space="PSUM") as ps_a:

                for name, xt in (("q", xqT), ("k", xkT), ("v", xvT)):
                    xt_t = xt.rearrange("(j p) s -> j p s", p=128)
                    xj = []
                    for j in range(6):
                        stg = xstage.tile([128, S], f32, tag="xstg")
                        nc.sync.dma_start(stg[:], xt_t[j])
                        xrj = xr.tile([128, S], f32r, tag=f"xr{j}")
                        nc.vector.tensor_copy(xrj[:], stg[:])
                        xj.append(xrj)

                    if name in ("q", "k"):
                        dstT = qT if name == "q" else kT
                        bias = bq_sb if name == "q" else bk_sb
                        wr = w_r[name]
                        for h in range(HPC):
                            for sc in range(4):  # 512-wide s chunks
                                p = ps_a.tile([64, 512], f32, tag="pqk")
                                for j in range(6):
                                    nc.tensor.matmul(
                                        p[:],
                                        wr[:, j * DL + h * DEPTH:
                                           j * DL + (h + 1) * DEPTH],
                                        xj[j][:, sc * 512:(sc + 1) * 512],
                                        start=(j == 0), stop=(j == 5))
                                nc.vector.tensor_scalar_add(
                                    dstT[h][:, sc * 512:(sc + 1) * 512],
                                    p[:], bias[:, h:h + 1])
                    else:
                        wr = w_r["v"]
                        for h in range(HPC):
                            for kc in range(NKC):  # 128-row s tiles
                                p = ps_a.tile([128, DEPTH], f32, tag="pv")
                                for j in range(6):
                                    nc.tensor.matmul(
                                        p[:],
                                        xj[j][:, kc * 128:(kc + 1) * 128],
                                        wr[:, j * DL + h * DEPTH:
                                           j * DL + (h + 1) * DEPTH],
                                        start=(j == 0), stop=(j == 5))
                                base = kc * (DEPTH + 1)
                                nc.vector.tensor_add(
                                    vaug[h][:, base:base + DEPTH], p[:],
                                    bv_bc[:, h * DEPTH:(h + 1) * DEPTH])
                                nc.vector.tensor_copy(
                                    vaug[h][:, base + DEPTH:base + DEPTH + 1],
                                    ones_col[:])

            # ================= Phase B: attention =================
            ctx_pool = ctx_stack.enter_context(tc.tile_pool(name="ctxT", bufs=1))
            ctxT = [ctx_pool.tile([64, S], f32r, tag=f"ctx{h}", name=f"ctx{h}")
                    for h in range(HPC)]
            with tc.tile_pool(name="pt", bufs=3) as pt_pool, \
                 tc.tile_pool(name="attn_sb", bufs=3) as attn_pool, \
                 tc.tile_pool(name="lnl", bufs=2) as lnl_pool, \
                 tc.tile_pool(name="mm_ps", bufs=2, space="PSUM") as mm_ps, \
                 tc.tile_pool(name="ctx_ps", bufs=2, space="PSUM") as ctx_ps:

                for h in range(HPC):
                    for qb in range(NQB):
                        q0 = qb * QB
                        # two 512-wide q halves accumulate separately
                        ctx_a = ctx_ps.tile([DEPTH + 1, 512], f32, tag="ctxps")
                        ctx_b = ctx_ps.tile([DEPTH + 1, 512], f32, tag="ctxps")
                        # ---- S^T pass: P^T chunks + AV accumulation ----
                        for kc in range(NKC):
                            st = mm_ps.tile([128, QB], f32, tag="mm")
                            for half in range(2):
                                nc.tensor.matmul(
                                    st[:, half * 512:(half + 1) * 512],
                                    kT[h][:, kc * 128:(kc + 1) * 128],
                                    qT[h][:, q0 + half * 512:q0 + (half + 1) * 512],
                                    start=True, stop=True)
                            pt = pt_pool.tile([128, QB], f32r, tag="pt")
                            nc.scalar.activation(pt[:], st[:], AF.Exp, scale=SCALE)
                            vslice = vaug[h][:, kc * (DEPTH + 1):
                                             (kc + 1) * (DEPTH + 1)]
                            nc.tensor.matmul(ctx_a[:], vslice, pt[:, 0:512],
                                             start=(kc == 0), stop=(kc == NKC - 1))
                            nc.tensor.matmul(ctx_b[:], vslice, pt[:, 512:QB],
                                             start=(kc == 0), stop=(kc == NKC - 1))

                        # ---- softmax denominators ----
                        # l row = last row of ctx_{a,b}; ln + transpose to cols
                        lnl_row = lnl_pool.tile([1, QB], f32, tag="lnlrow")
                        nc.scalar.activation(lnl_row[:, 0:512],
                                             ctx_a[DEPTH:DEPTH + 1, :], AF.Ln)
                        nc.scalar.activation(lnl_row[:, 512:QB],
                                             ctx_b[DEPTH:DEPTH + 1, :], AF.Ln)
                        nlnl_cols = lnl_pool.tile([128, 8], f32, tag="lnlcol")
                        for qt in range(8):
                            ptr = mm_ps.tile([128, 1], f32, tag="trp", bufs=1)
                            nc.tensor.transpose(
                                ptr[:], lnl_row[:, qt * 128:(qt + 1) * 128],
                                ident[:])
                            nc.vector.tensor_scalar_mul(
                                nlnl_cols[:, qt:qt + 1], ptr[:], -1.0)

                        # ---- normalize ctx^T ----
                        for half, cps in ((0, ctx_a), (1, ctx_b)):
                            lrow_r = lnl_pool.tile([1, 512], f32r, tag="lrowr")
                            nc.vector.tensor_copy(
                                lrow_r[:], cps[DEPTH:DEPTH + 1, :])
                            bc = mm_ps.tile([DEPTH, 512], f32, tag="bc", bufs=1)
                            nc.tensor.matmul(bc[:], ones_row[:, 0:DEPTH],
                                             lrow_r[:], start=True, stop=True)
                            rbc = lnl_pool.tile([DEPTH, 512], f32, tag="rbc")
                            nc.vector.reciprocal(rbc[:], bc[:])
                            nc.vector.tensor_mul(
                                ctxT[h][:, q0 + half * 512:q0 + (half + 1) * 512],
                                cps[0:DEPTH, :], rbc[:])

                        # ---- S pass: normalized attention rows ----
                        for qt in range(8):
                            r0 = q0 + qt * 128
                            asb = attn_pool.tile([128, S], f32, tag="attn")
                            for kb in range(2):  # 1024-wide k blocks
                                sp = mm_ps.tile([128, QB], f32, tag="mm")
                                for half in range(2):
                                    k0 = kb * QB + half * 512
                                    nc.tensor.matmul(
                                        sp[:, half * 512:(half + 1) * 512],
                                        qT[h][:, r0:r0 + 128],
                                        kT[h][:, k0:k0 + 512],
                                        start=True, stop=True)
                                nc.scalar.activation(
                                    asb[:, kb * QB:(kb + 1) * QB], sp[:],
                                    AF.Exp, scale=SCALE,
                                    bias=nlnl_cols[:, qt:qt + 1])
                            nc.sync.dma_start(attn_out[h, r0:r0 + 128, :], asb[:])

            # ================= Phase C: output projection =================
            with tc.tile_pool(name="osb", bufs=3) as osb_pool, \
                 tc.tile_pool(name="ps_o", bufs=2, space="PSUM") as ps_o:
                for st in range(NST):
                    po = ps_o.tile([128, D], f32, tag="po")
                    for h in range(HPC):
                        lhs = ctxT[h][:, st * 128:(st + 1) * 128]
                        nc.tensor.matmul(po[:, 0:512], lhs, wo_r[h][:, 0:512],
                                         start=(h == 0), stop=(h == HPC - 1))
                        nc.tensor.matmul(po[:, 512:768], lhs, wo_r[h][:, 512:768],
                                         start=(h == 0), stop=(h == HPC - 1))
                    osb = osb_pool.tile([128, D], f32, tag="osb")
                    nc.vector.tensor_add(osb[:], po[:], wob_bc[:])
                    nc.sync.dma_start(partial[st * 128:(st + 1) * 128, :], osb[:])

            ctx_stack.close()

            # ================= Phase D: reduce-scatter =================
            if skip_rs:
                return
            nc.gpsimd.collective_compute(
                "ReduceScatter", mybir.AluOpType.add,
                replica_groups=[[0, 1, 2, 3], [4, 5, 6, 7]],
                ins=[partial.opt()], outs=[rs_out.opt()])
            nc.sync.dma_start(out_slice[:], rs_out[:])

        if reps == 1:
            body()
        else:
            # collective inside a Tile For_i fails at runtime; loop phases
            # A-C only and do the reduce-scatter once at the end.
            with tc.For_i(0, reps, 1):
                body(skip_rs=True)
            nc.gpsimd.collective_compute(
                "ReduceScatter", mybir.AluOpType.add,
                replica_groups=[[0, 1, 2, 3], [4, 5, 6, 7]],
                ins=[partial.opt()], outs=[rs_out.opt()])
            nc.sync.dma_start(out_slice[:], rs_out[:])


def _numpy_fallback(q, k, v, mask, wq_w, wq_b, wk_w, wk_b, wv_w, wv_b,
                    wo_w, wo_b):
    def split_heads(x):
        b, s, _ = x.shape
        return x.reshape(b, s, H, DEPTH).transpose(0, 2, 1, 3)

    qh = split_heads(q @ wq_w.T + wq_b)
    kh = split_heads(k @ wk_w.T + wk_b)
    vh = split_heads(v @ wv_w.T + wv_b)
    logits = np.einsum('bhqd,bhkd->bhqk', qh, kh) / math.sqrt(DEPTH)
    logits = np.where(mask == 0, -np.inf, logits)
    m = logits.max(axis=-1, keepdims=True)
    e = np.exp(logits - m)
    attn = e / e.sum(axis=-1, keepdims=True)
    ctx = np.einsum('bhqk,bhkd->bhqd', attn, vh)
    concat = ctx.transpose(0, 2, 1, 3).reshape(q.shape[0], -1, D)
    out = concat @ wo_w.T + wo_b
    return out.astype(np.float32), attn.astype(np.float32)


def make_in_maps(q, k, v, wq_w, wq_b, wk_w, wk_b, wv_w, wv_b, wo_w, wo_b):
    in_maps = []
    xT = {}
    for b in range(B):
        xT[b] = (np.ascontiguousarray(q[b].T), np.ascontiguousarray(k[b].T),
                 np.ascontiguousarray(v[b].T))
    for c in range(N_CORES):
        b, g = c // 4, c % 4
        sl = slice(DL * g, DL * (g + 1))
        in_maps.append({
            "xqT": xT[b][0], "xkT": xT[b][1], "xvT": xT[b][2],
            "wqT": np.ascontiguousarray(wq_w[sl, :].T),
            "wkT": np.ascontiguousarray(wk_w[sl, :].T),
            "wvT": np.ascontiguousarray(wv_w[sl, :].T),
            "bq": np.ascontiguousarray(wq_b[sl]),
            "bk": np.ascontiguousarray(wk_b[sl]),
            "bv": np.ascontiguousarray(wv_b[sl]),
            "woT": np.ascontiguousarray(wo_w[:, sl].T),
            "wob": np.ascontiguousarray(wo_b * 0.25),
        })
    return in_maps


def kernel(q, k, v, mask, wq_w, wq_b, wk_w, wk_b, wv_w, wv_b, wo_w, wo_b):
    q = np.asarray(q, np.float32)
    k = np.asarray(k, np.float32)
    v = np.asarray(v, np.float32)
    mask = np.asarray(mask)
    wq_w = np.asarray(wq_w, np.float32); wq_b = np.asarray(wq_b, np.float32)
    wk_w = np.asarray(wk_w, np.float32); wk_b = np.asarray(wk_b, np.float32)
    wv_w = np.asarray(wv_w, np.float32); wv_b = np.asarray(wv_b, np.float32)
    wo_w = np.asarray(wo_w, np.float32); wo_b = np.asarray(wo_b, np.float32)

    if not np.all(mask != 0):
        return _numpy_fallback(q, k, v, mask, wq_w, wq_b, wk_w, wk_b,
                               wv_w, wv_b, wo_w, wo_b)

    if "prog" not in _PROGRAM_CACHE:
        _PROGRAM_CACHE["prog"] = build_program(reps=1)
    nc = _PROGRAM_CACHE["prog"]

    in_maps = make_in_maps(q, k, v, wq_w, wq_b, wk_w, wk_b, wv_w, wv_b,
                           wo_w, wo_b)
    res = run_bass_kernel_spmd(nc, in_maps, list(range(N_CORES)))

    attn = np.empty((B, H, S, S), np.float32)
    out = np.empty((B, S, D), np.float32)
    for c in range(N_CORES):
        b, g = c // 4, c % 4
        attn[b, HPC * g:HPC * (g + 1)] = res.results[c]["attn_out"]
        out[b, 512 * g:512 * (g + 1)] = res.results[c]["out_slice"]
    return out, attn


# revision 16
# speedup vs baseline: 10.0871x; 10.0871x over previous
"""Trainium2 Bass kernel for nn_MultiHeadAttention (B=2, S=2048, D=768, H=12).

Sharding: (batch, head-group) across 8 cores — core c handles batch c//4 and
heads 3*(c%4) .. 3*(c%4)+2 (Megatron-style column/row-parallel projections).
Returns (out [2,2048,768], attn [2,12,2048,2048]) like the reference.

Structure (single flat pool scope so phases overlap freely):
  A: stream x^T in 1024-col halves; project to Q^T/K^T (fp32r) and
     V (+ones column, bf16).
  B: per (q-block, head): S^T matmuls -> ACT exp -> P^T (bf16) -> AV
     accumulation [65, 512] x2 whose last row is the softmax denominator l;
     ln(l) -> per-partition bias so the S pass emits final normalized
     attention rows directly via exp(S/8 - ln l); ctx^T normalized by 1/l
     (broadcast via outer-product matmul).  O-projection runs per q-block.
  C: ReduceScatter(+) of the partial output over groups of 4 cores.
"""

import math
import numpy as np

import concourse.bass as bass
import concourse.tile as tile
from concourse import bacc, mybir
from concourse.bass_utils import run_bass_kernel_spmd

B, S, D, H = 2, 2048, 768, 12
DEPTH = D // H          # 64
N_CORES = 8
HPC = H // 4            # 3 heads per core
DL = HPC * DEPTH        # 192 local head dims per core
QB = 1024               # q-block width
NQB = S // QB           # 2
NKC = S // 128          # 16 k-chunks
SCALE = 1.0 / math.sqrt(DEPTH)  # 0.125
VW = DEPTH + 1          # 65: V columns + ones

dt = mybir.dt
AF = mybir.ActivationFunctionType

_PROGRAM_CACHE = {}


def build_program(reps: int = 1, single_core: bool = False, parts: str = "abc",
                  timing_mode: bool = False):
    nc = bacc.Bacc("TRN2", target_bir_lowering=False, debug=False,
                   num_devices=1 if single_core else N_CORES)
    f32 = dt.float32

    xqT = nc.dram_tensor("xqT", [D, S], f32, kind="ExternalInput").ap()
    xkT = nc.dram_tensor("xkT", [D, S], f32, kind="ExternalInput").ap()
    xvT = nc.dram_tensor("xvT", [D, S], f32, kind="ExternalInput").ap()
    wqT = nc.dram_tensor("wqT", [D, DL], f32, kind="ExternalInput").ap()
    wkT = nc.dram_tensor("wkT", [D, DL], f32, kind="ExternalInput").ap()
    wvT = nc.dram_tensor("wvT", [D, DL], f32, kind="ExternalInput").ap()
    bq = nc.dram_tensor("bq", [DL], f32, kind="ExternalInput").ap()
    bk = nc.dram_tensor("bk", [DL], f32, kind="ExternalInput").ap()
    bv = nc.dram_tensor("bv", [DL], f32, kind="ExternalInput").ap()
    woT = nc.dram_tensor("woT", [DL, D], f32, kind="ExternalInput").ap()
    wob = nc.dram_tensor("wob", [D], f32, kind="ExternalInput").ap()

    if timing_mode:
        attn_out = nc.dram_tensor("attn_out", [HPC, S, S], f32).ap()
        out_slice = nc.dram_tensor("out_slice", [S // 4, D], f32).ap()
        dummy = nc.dram_tensor("timing_out", [1, 1], f32,
                               kind="ExternalOutput").ap()
    else:
        attn_out = nc.dram_tensor("attn_out", [HPC, S, S], f32,
                                  kind="ExternalOutput").ap()
        out_slice = nc.dram_tensor("out_slice", [S // 4, D], f32,
                                   kind="ExternalOutput").ap()
        dummy = None

    with tile.TileContext(nc) as tc:
        _trace_body(nc, tc, xqT, xkT, xvT, wqT, wkT, wvT, bq, bk, bv,
                    woT, wob, attn_out, out_slice, reps,
                    no_rs=single_core, parts=parts, dummy=dummy)

    nc.compile()
    return nc


def _trace_body(nc, tc, xqT, xkT, xvT, wqT, wkT, wvT, bq, bk, bv,
                woT, wob, attn_out, out_slice, reps, no_rs=False,
                parts: str = "abc", dummy=None):
    f32, f32r, bf16 = dt.float32, dt.float32r, dt.bfloat16

    from contextlib import ExitStack
    es = ExitStack()
    with es:
        const_pool = es.enter_context(tc.tile_pool(name="const", bufs=1))
        wpool = es.enter_context(tc.tile_pool(name="weights", bufs=1))
        qk_pool = es.enter_context(tc.tile_pool(name="qkT", bufs=1))
        v_pool = es.enter_context(tc.tile_pool(name="vaug", bufs=1))
        ctx_pool = es.enter_context(tc.tile_pool(name="ctxT", bufs=1))
        xstage = es.enter_context(tc.tile_pool(name="xstage", bufs=2))
        xr_pool = es.enter_context(tc.tile_pool(name="xr", bufs=1))
        pt_pool = es.enter_context(tc.tile_pool(name="pt", bufs=3))
        attn_pool = es.enter_context(tc.tile_pool(name="attn_sb", bufs=3))
        lnl_pool = es.enter_context(tc.tile_pool(name="lnl", bufs=2))
        osb_pool = es.enter_context(tc.tile_pool(name="osb", bufs=2))
        dram = es.enter_context(tc.tile_pool(name="dram", bufs=1, space="DRAM"))
        mm_ps = es.enter_context(
            tc.tile_pool(name="mm_ps", bufs=3, space="PSUM"))
        ctx_ps = es.enter_context(
            tc.tile_pool(name="ctx_ps", bufs=2, space="PSUM"))

        # ---------- constants ----------
        ones_col32 = const_pool.tile([128, 1], f32)
        nc.gpsimd.memset(ones_col32[:], 1.0)
        ones_col = const_pool.tile([128, 1], f32r)
        nc.vector.tensor_copy(ones_col[:], ones_col32[:])
        ones_row32 = const_pool.tile([1, 128], f32)
        nc.gpsimd.memset(ones_row32[:], 1.0)
        ones_row = const_pool.tile([1, 128], f32r)
        nc.vector.tensor_copy(ones_row[:], ones_row32[:])
        ident = const_pool.tile([1, 1], f32)
        nc.gpsimd.memset(ident[:], 1.0)

        # ---------- weights ----------
        w_r = {}
        wo_r = []
        with tc.tile_pool(name="wstage", bufs=1) as wstage:
            for name, wt in (("q", wqT), ("k", wkT), ("v", wvT)):
                wt_t = wt.rearrange("(j p) d -> j p d", p=128)
                big = wpool.tile([128, 6 * DL], f32r, tag=f"w{name}")
                for j in range(6):
                    stg = wstage.tile([128, DL], f32, tag="wstg")
                    nc.sync.dma_start(stg[:], wt_t[j])
                    nc.vector.tensor_copy(big[:, j * DL:(j + 1) * DL], stg[:])
                w_r[name] = big
            woT_t = woT.rearrange("(h p) d -> h p d", p=DEPTH)
            for h in range(HPC):
                stg = wstage.tile([64, D], f32, tag="wostg")
                nc.sync.dma_start(stg[:], woT_t[h])
                t = wpool.tile([64, D], f32r, tag=f"wo{h}")
                nc.vector.tensor_copy(t[:], stg[:])
                wo_r.append(t)
        bq_sb = const_pool.tile([64, HPC], f32)
        bk_sb = const_pool.tile([64, HPC], f32)
        nc.sync.dma_start(bq_sb[:], bq.rearrange("(h d) -> d h", d=DEPTH))
        nc.sync.dma_start(bk_sb[:], bk.rearrange("(h d) -> d h", d=DEPTH))
        brow = const_pool.tile([1, DL], f32)
        nc.sync.dma_start(brow[:], bv[None, :])
        brow_r = const_pool.tile([1, DL], f32r)
        nc.vector.tensor_copy(brow_r[:], brow[:])
        wrow = const_pool.tile([1, D], f32)
        nc.sync.dma_start(wrow[:], wob[None, :])
        wrow_r = const_pool.tile([1, D], f32r)
        nc.vector.tensor_copy(wrow_r[:], wrow[:])

        bv_bc = const_pool.tile([128, DL], f32)
        wob_bc = const_pool.tile([128, D], f32)
        p = mm_ps.tile([128, DL], f32, tag="mm")
        nc.tensor.matmul(p[:], ones_row[:], brow_r[:], start=True, stop=True)
        nc.vector.tensor_copy(bv_bc[:], p[:])
        p2 = mm_ps.tile([128, D], f32, tag="mm")
        nc.tensor.matmul(p2[:, 0:512], ones_row[:], wrow_r[:, 0:512],
                         start=True, stop=True)
        nc.tensor.matmul(p2[:, 512:768], ones_row[:], wrow_r[:, 512:768],
                         start=True, stop=True)
        nc.vector.tensor_copy(wob_bc[:], p2[:])

        # ---------- persistent data tiles ----------
        qT = [qk_pool.tile([64, S], f32r, tag=f"qT{h}", name=f"qT{h}")
              for h in range(HPC)]
        kT = [qk_pool.tile([64, S], f32r, tag=f"kT{h}", name=f"kT{h}")
              for h in range(HPC)]
        vaug = [v_pool.tile([128, NKC * VW], f32r, tag=f"va{h}", name=f"va{h}")
                for h in range(HPC)]
        ctxT = [ctx_pool.tile([64, S], f32r, tag=f"ctx{h}", name=f"ctx{h}")
                for h in range(HPC)]

        partial = dram.tile([S, D], f32)
        rs_out = dram.tile([S // 4, D], f32)

        if dummy is not None:
            dsb = const_pool.tile([1, 1], f32)
            nc.gpsimd.memset(dsb[:], 1.0)
            nc.sync.dma_start(dummy[:], dsb[:])

        def phase_a():
            xsrc = {"q": xqT, "k": xkT, "v": xvT}
            for half in range(2):
                h0 = half * QB
                for name in ("k", "q", "v"):
                    xt_t = xsrc[name].rearrange("(j p) s -> j p s", p=128)
                    xj = []
                    for j in range(6):
                        stg = xstage.tile([128, QB], f32, tag="xstg")
                        nc.sync.dma_start(stg[:], xt_t[j][:, h0:h0 + QB])
                        xrj = xr_pool.tile([128, QB], f32r, tag=f"xr{j}",
                                           name=f"xr{j}")
                        nc.vector.tensor_copy(xrj[:], stg[:])
                        xj.append(xrj)

                    if name in ("q", "k"):
                        dstT = qT if name == "q" else kT
                        bias = bq_sb if name == "q" else bk_sb
                        wr = w_r[name]
                        for h in range(HPC):
                            for sc in range(2):  # 512-wide chunks in half
                                pj = mm_ps.tile([64, 512], f32, tag="mm")
                                for j in range(6):
                                    nc.tensor.matmul(
                                        pj[:],
                                        wr[:, j * DL + h * DEPTH:
                                           j * DL + (h + 1) * DEPTH],
                                        xj[j][:, sc * 512:(sc + 1) * 512],
                                        start=(j == 0), stop=(j == 5))
                                nc.vector.tensor_scalar_add(
                                    dstT[h][:, h0 + sc * 512:
                                            h0 + (sc + 1) * 512],
                                    pj[:], bias[:, h:h + 1])
                    else:
                        wr = w_r["v"]
                        for h in range(HPC):
                            for st in range(8):  # 128-row s tiles in half
                                pj = mm_ps.tile([128, DEPTH], f32, tag="mm")
                                for j in range(6):
                                    nc.tensor.matmul(
                                        pj[:],
                                        xj[j][:, st * 128:(st + 1) * 128],
                                        wr[:, j * DL + h * DEPTH:
                                           j * DL + (h + 1) * DEPTH],
                                        start=(j == 0), stop=(j == 5))
                                kc = half * 8 + st
                                base = kc * VW
                                nc.vector.tensor_add(
                                    vaug[h][:, base:base + DEPTH], pj[:],
                                    bv_bc[:, h * DEPTH:(h + 1) * DEPTH])
                                nc.vector.tensor_copy(
                                    vaug[h][:, base + DEPTH:base + VW],
                                    ones_col[:])

        def phase_b():
            for qb in range(NQB):
                q0 = qb * QB
                for h in range(HPC):
                    ctx_a = ctx_ps.tile([VW, 512], f32, tag="ctxps")
                    ctx_b = ctx_ps.tile([VW, 512], f32, tag="ctxps")
                    # ---- S^T pass: P^T chunks + AV accumulation ----
                    for kc in range(NKC):
                        st = mm_ps.tile([128, QB], f32, tag="mm")
                        for hf in range(2):
                            nc.tensor.matmul(
                                st[:, hf * 512:(hf + 1) * 512],
                                kT[h][:, kc * 128:(kc + 1) * 128],
                                qT[h][:, q0 + hf * 512:q0 + (hf + 1) * 512],
                                start=True, stop=True)
                        pt = pt_pool.tile([128, QB], f32r, tag="pt")
                        nc.scalar.activation(pt[:], st[:], AF.Exp, scale=SCALE)
                        vsl = vaug[h][:, kc * VW:(kc + 1) * VW]
                        nc.tensor.matmul(ctx_a[:], vsl, pt[:, 0:512],
                                         start=(kc == 0), stop=(kc == NKC - 1))
                        nc.tensor.matmul(ctx_b[:], vsl, pt[:, 512:QB],
                                         start=(kc == 0), stop=(kc == NKC - 1))

                    # ---- normalize ctx^T ----
                    for hf, cps in ((0, ctx_a), (1, ctx_b)):
                        lrow_r = lnl_pool.tile([1, 512], f32r, tag="lrowr")
                        nc.vector.tensor_copy(lrow_r[:], cps[DEPTH:VW, :])
                        bcp = mm_ps.tile([DEPTH, 512], f32, tag="mm")
                        nc.tensor.matmul(bcp[:], ones_row[:, 0:DEPTH],
                                         lrow_r[:], start=True, stop=True)
                        rbc = lnl_pool.tile([DEPTH, 512], f32, tag="rbc")
                        nc.vector.reciprocal(rbc[:], bcp[:])
                        nc.vector.tensor_mul(
                            ctxT[h][:, q0 + hf * 512:q0 + (hf + 1) * 512],
                            cps[0:DEPTH, :], rbc[:])

                    # ---- S pass: attention rows (exp + DVE normalize) ----
                    for qt in range(8):
                        r0 = q0 + qt * 128
                        asb = attn_pool.tile([128, S], f32, tag="attn")
                        lpart = lnl_pool.tile([128, 2], f32, tag="lpart")
                        for kb in range(2):
                            sp = mm_ps.tile([128, QB], f32, tag="mm")
                            for hf in range(2):
                                k0 = kb * QB + hf * 512
                                nc.tensor.matmul(
                                    sp[:, hf * 512:(hf + 1) * 512],
                                    qT[h][:, r0:r0 + 128],
                                    kT[h][:, k0:k0 + 512],
                                    start=True, stop=True)
                            nc.scalar.activation(
                                asb[:, kb * QB:(kb + 1) * QB], sp[:],
                                AF.Exp, scale=SCALE,
                                accum_out=lpart[:, kb:kb + 1])
                        lsum = lnl_pool.tile([128, 1], f32, tag="lsum")
                        nc.vector.reduce_sum(lsum[:], lpart[:],
                                             axis=mybir.AxisListType.X)
                        rcol = lnl_pool.tile([128, 1], f32, tag="rcol")
                        nc.vector.reciprocal(rcol[:], lsum[:])
                        nc.vector.tensor_scalar_mul(asb[:], asb[:], rcol[:])
                        if "noadma" not in parts:
                            nc.sync.dma_start(attn_out[h, r0:r0 + 128, :],
                                              asb[:])

                # ---- O-projection for this q-block ----
                if "c" in parts:
                    for st in range(8):
                        po = mm_ps.tile([128, D], f32, tag="mm")
                        for h in range(HPC):
                            lhs = ctxT[h][:, q0 + st * 128:q0 + (st + 1) * 128]
                            nc.tensor.matmul(po[:, 0:512], lhs,
                                             wo_r[h][:, 0:512],
                                             start=(h == 0),
                                             stop=(h == HPC - 1))
                            nc.tensor.matmul(po[:, 512:768], lhs,
                                             wo_r[h][:, 512:768],
                                             start=(h == 0),
                                             stop=(h == HPC - 1))
                        osb = osb_pool.tile([128, D], f32, tag="osb")
                        nc.vector.tensor_add(osb[:], po[:], wob_bc[:])
                        nc.sync.dma_start(
                            partial[q0 + st * 128:q0 + (st + 1) * 128, :],
                            osb[:])

        def body():
            phase_a()
            if "b" in parts:
                phase_b()

        if reps == 1:
            body()
        else:
            with tc.For_i(0, reps, 1):
                body()

        # ---- reduce-scatter (once; collectives break inside For_i) ----
        if not no_rs and "c" in parts:
            nc.gpsimd.collective_compute(
                "ReduceScatter", mybir.AluOpType.add,
                replica_groups=[[0, 1, 2, 3], [4, 5, 6, 7]],
                ins=[partial.opt()], outs=[rs_out.opt()])
            nc.sync.dma_start(out_slice[:], rs_out[:])


def _numpy_fallback(q, k, v, mask, wq_w, wq_b, wk_w, wk_b, wv_w, wv_b,
                    wo_w, wo_b):
    def split_heads(x):
        b, s, _ = x.shape
        return x.reshape(b, s, H, DEPTH).transpose(0, 2, 1, 3)

    qh = split_heads(q @ wq_w.T + wq_b)
    kh = split_heads(k @ wk_w.T + wk_b)
    vh = split_heads(v @ wv_w.T + wv_b)
    logits = np.einsum('bhqd,bhkd->bhqk', qh, kh) / math.sqrt(DEPTH)
    logits = np.where(mask == 0, -np.inf, logits)
    m = logits.max(axis=-1, keepdims=True)
    e = np.exp(logits - m)
    attn = e / e.sum(axis=-1, keepdims=True)
    ctx = np.einsum('bhqk,bhkd->bhqd', attn, vh)
    concat = ctx.transpose(0, 2, 1, 3).reshape(q.shape[0], -1, D)
    out = concat @ wo_w.T + wo_b
    return out.astype(np.float32), attn.astype(np.float32)


def make_in_maps(q, k, v, wq_w, wq_b, wk_w, wk_b, wv_w, wv_b, wo_w, wo_b):
    in_maps = []
    xT = {}
    for b in range(B):
        xT[b] = (np.ascontiguousarray(q[b].T), np.ascontiguousarray(k[b].T),
                 np.ascontiguousarray(v[b].T))
    for c in range(N_CORES):
        b, g = c // 4, c % 4
        sl = slice(DL * g, DL * (g + 1))
        in_maps.append({
            "xqT": xT[b][0], "xkT": xT[b][1], "xvT": xT[b][2],
            "wqT": np.ascontiguousarray(wq_w[sl, :].T),
            "wkT": np.ascontiguousarray(wk_w[sl, :].T),
            "wvT": np.ascontiguousarray(wv_w[sl, :].T),
            "bq": np.ascontiguousarray(wq_b[sl]),
            "bk": np.ascontiguousarray(wk_b[sl]),
            "bv": np.ascontiguousarray(wv_b[sl]),
            "woT": np.ascontiguousarray(wo_w[:, sl].T),
            "wob": np.ascontiguousarray(wo_b * 0.25),
        })
    return in_maps


def kernel(q, k, v, mask, wq_w, wq_b, wk_w, wk_b, wv_w, wv_b, wo_w, wo_b):
    q = np.asarray(q, np.float32)
    k = np.asarray(k, np.float32)
    v = np.asarray(v, np.float32)
    mask = np.asarray(mask)
    wq_w = np.asarray(wq_w, np.float32); wq_b = np.asarray(wq_b, np.float32)
    wk_w = np.asarray(wk_w, np.float32); wk_b = np.asarray(wk_b, np.float32)
    wv_w = np.asarray(wv_w, np.float32); wv_b = np.asarray(wv_b, np.float32)
    wo_w = np.asarray(wo_w, np.float32); wo_b = np.asarray(wo_b, np.float32)

    if not np.all(mask != 0):
        return _numpy_fallback(q, k, v, mask, wq_w, wq_b, wk_w, wk_b,
                               wv_w, wv_b, wo_w, wo_b)

    if "prog" not in _PROGRAM_CACHE:
        _PROGRAM_CACHE["prog"] = build_program(reps=1)
    nc = _PROGRAM_CACHE["prog"]

    in_maps = make_in_maps(q, k, v, wq_w, wq_b, wk_w, wk_b, wv_w, wv_b,
                           wo_w, wo_b)
    res = run_bass_kernel_spmd(nc, in_maps, list(range(N_CORES)))

    attn = np.empty((B, H, S, S), np.float32)
    out = np.empty((B, S, D), np.float32)
    for c in range(N_CORES):
        b, g = c // 4, c % 4
        attn[b, HPC * g:HPC * (g + 1)] = res.results[c]["attn_out"]
        out[b, 512 * g:512 * (g + 1)] = res.results[c]["out_slice"]
    return out, attn
